# revision 14
# baseline (speedup 1.0000x reference)
"""Trainium2 Bass kernel v8 for nn_MultiHeadAttention (B=2, S=2048, D=1024, H=16).

Sharding: 8 cores = 2 batches x 4 head-groups (4 heads each).
Host folds the per-(batch,head) sigmoid gate into Wo rows (linear in the
head outputs), so no gate math on-chip.

v8 schedule (measured on HW traces):
  - attention steps (scores -> exp -> AV) stream at the ACT exp pace;
    all projection / out-projection work is decomposed into ~0.9us
    filler units popped one-per-step between attention steps so the PE
    never idles (keeps the HAM clock warm) and never backs up in front
    of the next scores matmul;
  - K tiles are stored zero-padded to 128 contraction rows per head
    (ktp_e rows 64-127 = 0, ktp_o rows 0-63 = 0) so score matmuls are
    full-array: their LDWEIGHTS hides behind the previous matmul's
    stream (row-group-tiled 64-row matmuls pay LDWEIGHTS serially,
    measured +107ns on every score matmul);
  - causal mask via a bf16 DVE multiply on the exp'd scores (2x mode);
  - softmax denominators: per-hp Ln + Exp(-1) (same ACT table set as
    the softmax exp - keep them together or tables thrash), and the
    reciprocal row is partition-broadcast by a stride-0 DMA instead of
    a ones-matmul (no PE work, no shuffle DMAs);
  - x chunks DMA-prefetched a full iteration ahead (per-k pieces,
    interleaved with weights for chunk 0); output stores and hcat
    moves issue from the gpsimd DGE queue to keep the sync queue free
    for the prefetch stream.

Host sums the 4 bf16 head-group partials per batch (fp32) and adds bo.
"""

import numpy as np

P = 128
CHUNK = 512

_BUILD_CACHE = {}


def _build(S, D, DOUT, HPC, DK, causal, with_bias):
    import concourse.bass as bass
    import concourse.mybir as mybir
    import concourse.tile as tile
    from concourse import bacc
    from concourse.bass import ds, ts

    fp32 = mybir.dt.float32
    bf16 = mybir.dt.bfloat16
    KC = D // P             # contraction k-chunks for projections
    GCOLS = HPC * DK        # this core's projection output width
    MT = GCOLS // P         # head-pair tiles (2 heads of DK=64 per tile)
    NCH = S // CHUNK        # q-chunks
    TPC = CHUNK // P        # kv tiles per q-chunk (4)
    NKV = S // P            # kv tiles total
    KC2 = GCOLS // P        # out-proj contraction chunks
    NOC = DOUT // CHUNK     # out-proj N chunks
    HC = CHUNK // 2         # stage-A unit free size
    assert DK * 2 == P and GCOLS % P == 0

    Act = mybir.ActivationFunctionType
    nc = bacc.Bacc()

    # Pin Exp/Ln to the combined table set: the placement pass otherwise
    # alternates exp-only and ln-only sets, reloading tables (~2.7us + ACT
    # pipeline stall) around every softmax-denominator normalization.
    from concourse.hw_specs import get_activation_tables
    tables = get_activation_tables(nc.m.arch)
    if "natural_log_exp_and_others" in tables:
        for name, fns in tables.items():
            if name != "natural_log_exp_and_others":
                fns.discard(Act.Exp)
                fns.discard(Act.Ln)

    xqT = nc.declare_dram_parameter("xqT", [D, S], bf16, isOutput=False)
    xkT = nc.declare_dram_parameter("xkT", [D, S], bf16, isOutput=False)
    xvT = nc.declare_dram_parameter("xvT", [D, S], bf16, isOutput=False)
    wq_d = nc.declare_dram_parameter("wq", [D, GCOLS], bf16, isOutput=False)
    wk_d = nc.declare_dram_parameter("wk", [D, GCOLS], bf16, isOutput=False)
    wv_d = nc.declare_dram_parameter("wv", [D, GCOLS], bf16, isOutput=False)
    wo_d = nc.declare_dram_parameter("wo", [GCOLS, DOUT], bf16, isOutput=False)
    if with_bias:
        bq_d = nc.declare_dram_parameter("bq", [GCOLS], fp32, isOutput=False)
        bk_d = nc.declare_dram_parameter("bk", [GCOLS], fp32, isOutput=False)
        bv_d = nc.declare_dram_parameter("bv", [1, GCOLS], bf16, isOutput=False)
    mtri_d = nc.declare_dram_parameter("mtri", [P, 2, P], bf16, isOutput=False)
    outp = nc.declare_dram_parameter("out", [S, DOUT], bf16, isOutput=True)

    scale = 1.0 / float(np.sqrt(DK))

    with tile.TileContext(nc) as tc:
        with (
            tc.tile_pool(name="persist", bufs=1) as pp,
            tc.tile_pool(name="wts", bufs=1) as wp,
            tc.tile_pool(name="xsub", bufs=6) as xp,
            tc.tile_pool(name="attn", bufs=3) as ap_,
            tc.tile_pool(name="avsb", bufs=2) as avp,
            tc.tile_pool(name="rows", bufs=2) as rp,
            tc.tile_pool(name="otmp", bufs=2) as op_,
            tc.tile_pool(name="osb", bufs=3) as ob,
            tc.tile_pool(name="psmm", bufs=2, space="PSUM") as psmm,
            tc.tile_pool(name="pssc", bufs=2, space="PSUM") as pssc,
            tc.tile_pool(name="psav", bufs=2, space="PSUM") as psav,
        ):
            qt = pp.tile([P, MT, S], bf16, tag="qt")
            # K stored zero-padded per head half: full-128-row lhsT for
            # the score matmuls (LDWEIGHTS hides; see module docstring).
            kte = pp.tile([P, MT, S], bf16, tag="kte")
            kto = pp.tile([P, MT, S], bf16, tag="kto")
            vaug = pp.tile([P, NKV, HPC, DK + 1], bf16, tag="vaug")
            hcat = pp.tile([P, KC2, S], bf16, tag="hcat")
            ones_bf = pp.tile([1, P], bf16, tag="ones_bf")
            ones_row = pp.tile([1, CHUNK], bf16, tag="ones_row")
            nc.any.memset(ones_bf[:], 1.0)
            nc.any.memset(ones_row[:], 1.0)
            nc.any.memset(vaug[:, :, :, DK : DK + 1], 1.0)
            nc.any.memset(kte[DK:P, :, :], 0.0)
            nc.any.memset(kto[0:DK, :, :], 0.0)

            xq_t = xqT.rearrange("(c p) s -> p c s", p=P)
            xk_t = xkT.rearrange("(c p) s -> p c s", p=P)
            xv_t = xvT.rearrange("(c p) s -> p c s", p=P)

            # ---------------- DMA prefetch (per-k pieces; K, Q first)
            xpref = {}

            def prefetch_x(n):
                if n >= NCH:
                    return
                nsl = ds(n * CHUNK, CHUNK)
                tl = {}
                for nm, src in (("k", xk_t), ("q", xq_t), ("v", xv_t)):
                    t_ = xp.tile([P, KC, CHUNK], bf16, tag="xsub",
                                 name="xsub", bufs=6)
                    for k in range(KC):
                        nc.sync.dma_start(t_[:, k, :], src[:, k, nsl])
                    tl[nm] = t_
                xpref[n] = tl

            # chunk-0 loads interleave each x k-piece with its weight
            # k-piece so the first projection chain starts early.
            wk = wp.tile([P, KC, GCOLS], bf16, tag="wk")
            wq = wp.tile([P, KC, GCOLS], bf16, tag="wq")
            wv = wp.tile([P, KC, GCOLS], bf16, tag="wv")
            wk_r = wk_d.rearrange("(c p) n -> p c n", p=P)
            wq_r = wq_d.rearrange("(c p) n -> p c n", p=P)
            wv_r = wv_d.rearrange("(c p) n -> p c n", p=P)
            x0 = {}
            for nm in ("k", "q", "v"):
                t_ = xp.tile([P, KC, CHUNK], bf16, tag="xsub",
                             name="xsub", bufs=6)
                x0[nm] = t_
            for nm, src, w_sb, w_r in (("k", xk_t, wk, wk_r),
                                       ("q", xq_t, wq, wq_r),
                                       ("v", xv_t, wv, wv_r)):
                for k in range(KC):
                    nc.sync.dma_start(x0[nm][:, k, :], src[:, k, ds(0, CHUNK)])
                    nc.sync.dma_start(w_sb[:, k, :], w_r[:, k, :])
                if nm == "q":
                    mtri = wp.tile([P, 2, P], bf16, tag="mtri")
                    nc.sync.dma_start(mtri[:], mtri_d[:])
            xpref[0] = x0
            if with_bias:
                bq = wp.tile([P, MT], fp32, tag="bq")
                bk = wp.tile([P, MT], fp32, tag="bk")
                nc.sync.dma_start(bq[:], bq_d.rearrange("(m p) -> p m", p=P))
                nc.sync.dma_start(bk[:], bk_d.rearrange("(m p) -> p m", p=P))
                bv = wp.tile([1, GCOLS], bf16, tag="bv")
                nc.sync.dma_start(bv[:], bv_d[:])
            prefetch_x(1)
            wo = wp.tile([P, KC2, DOUT], bf16, tag="wo")
            nc.sync.dma_start(wo[:], wo_d.rearrange("(c p) n -> p c n", p=P))

            # ---------------- PE pre-warm during the DMA-bound ramp
            def dummy_mms(cnt):
                for _ in range(cnt):
                    ps = psmm.tile([P, CHUNK], fp32, tag="psa", bufs=2)
                    nc.tensor.matmul(ps[:], ones_bf[0:1, :], ones_row[:],
                                     start=True, stop=True)

            # ---------------- stage A (QKV projections) as filler units
            def make_stage_a_units(n):
                tl = xpref.pop(n)
                xsk, xsq, xsv = tl["k"], tl["q"], tl["v"]
                units = []

                def proju(xs_, w_sb, bname, m, ch):
                    def u():
                        csl = ds(ch * HC, HC)
                        osl = ds(n * CHUNK + ch * HC, HC)
                        ps = psmm.tile([P, CHUNK], fp32, tag="psa", bufs=2)
                        for k in range(KC):
                            nc.tensor.matmul(
                                ps[:, 0:HC], w_sb[:, k, ts(m, P)],
                                xs_[:, k, csl],
                                start=(k == 0), stop=(k == KC - 1))
                        if bname == "bq":
                            if with_bias:
                                nc.vector.tensor_scalar_add(
                                    qt[:, m, osl], ps[:, 0:HC],
                                    bq[:, m : m + 1])
                            else:
                                nc.vector.tensor_copy(qt[:, m, osl],
                                                      ps[:, 0:HC])
                        else:
                            # zero-padded K halves (pad rows memset once)
                            if with_bias:
                                nc.vector.tensor_scalar_add(
                                    kte[0:DK, m, osl], ps[0:DK, 0:HC],
                                    bk[0:DK, m : m + 1])
                                nc.vector.tensor_scalar_add(
                                    kto[DK:P, m, osl], ps[DK:P, 0:HC],
                                    bk[DK:P, m : m + 1])
                            else:
                                nc.vector.tensor_copy(kte[0:DK, m, osl],
                                                      ps[0:DK, 0:HC])
                                nc.vector.tensor_copy(kto[DK:P, m, osl],
                                                      ps[DK:P, 0:HC])
                    return u

                for xs_, w_sb, bname in ((xsk, wk, "bk"), (xsq, wq, "bq")):
                    for m in range(MT):
                        for ch in (0, 1):
                            units.append(proju(xs_, w_sb, bname, m, ch))

                def vu(st2):
                    def u():
                        st = n * TPC + st2
                        ps = psmm.tile([P, CHUNK], fp32, tag="psa", bufs=2)
                        last_v = KC - 1 if not with_bias else -1
                        for k in range(KC):
                            nc.tensor.matmul(
                                ps[:, :GCOLS], xsv[:, k, ts(st2, P)],
                                wv[:, k, :], start=(k == 0),
                                stop=(k == last_v))
                        if with_bias:
                            nc.tensor.matmul(
                                ps[:, :GCOLS], ones_bf[0:1, 0:P], bv[:],
                                start=False, stop=True)
                        nc.vector.tensor_copy(
                            vaug[:, st, :, 0:DK],
                            ps[:, :GCOLS].rearrange("p (h d) -> p h d", d=DK))
                    return u

                vunits = [vu(st2) for st2 in range(TPC)]
                return units, vunits

            # ---------------- attention step helpers
            state = {}

            def issue_scores(j, hp, i):
                t = i - TPC * j
                diag = causal and t >= 0
                coff = P * t if diag else 0
                qoff = j * CHUNK + coff
                Ni = CHUNK - coff
                psp = pssc.tile([P, 2, CHUNK], fp32, name="sc", tag="sc",
                                bufs=2)
                for half, ktp in ((0, kte), (1, kto)):
                    nc.tensor.matmul(
                        psp[:, half, coff:], ktp[:, hp, ts(i, P)],
                        qt[:, hp, ds(qoff, Ni)], start=True, stop=True)
                state[(hp, i)] = (psp, coff, Ni, diag)

            def issue_exp(hp, i):
                psp, coff, Ni, diag = state[(hp, i)]
                at = ap_.tile([P, 2, CHUNK], bf16, tag="at")
                nc.scalar.activation(at[:, :, coff:], psp[:, :, coff:],
                                     Act.Exp, scale=scale)
                if diag:
                    nc.vector.tensor_mul(
                        at[:, :, coff : coff + P],
                        at[:, :, coff : coff + P], mtri[:])
                state[(hp, i)] = (psp, coff, Ni, at)

            def issue_av(hp, i, first, last, pe, po):
                _, coff, Ni, at = state.pop((hp, i))
                for half, pav in ((0, pe), (1, po)):
                    nc.tensor.matmul(
                        pav[:, ds(coff, Ni)], vaug[:, i, 2 * hp + half, :],
                        at[:, half, coff:], start=first, stop=last)

            # ---------------- normalize
            # denominators live as [2, MT, CHUNK]: partition = half (ACT
            # partition base always 0), free block = head-pair.
            sbs = {}
            dens = {}
            rsts = {}

            def phase1(j, hp, pe, po):
                av_sb = avp.tile([DK + 1, 2, CHUNK], fp32, tag="avsb")
                nc.vector.tensor_copy(av_sb[:, 0, :], pe[:])
                nc.vector.tensor_copy(av_sb[:, 1, :], po[:])
                sbs[(j, hp)] = av_sb
                if hp == 0:
                    den2_t = rp.tile([2, MT, CHUNK], fp32, tag="den2",
                                     name="den2")
                    rr2_t = rp.tile([2, MT, CHUNK], bf16, tag="rr2",
                                    name="rr2")
                    dens[j] = (den2_t, rr2_t)
                den2, rr2 = dens[j]
                for half in (0, 1):
                    nc.sync.dma_start(den2[half : half + 1, hp, :],
                                      av_sb[DK : DK + 1, half, :])
                nc.scalar.activation(den2[0:2, hp, :], den2[0:2, hp, :],
                                     Act.Ln)
                nc.scalar.activation(rr2[0:2, hp, :], den2[0:2, hp, :],
                                     Act.Exp, scale=-1.0)
                rst = rp.tile([1, CHUNK], bf16, tag=f"rst{hp}", name="rst")
                nc.sync.dma_start(rst[:], rr2[1:2, hp, :])
                rsts[(j, hp)] = rst

            def phase2(j, hp):
                # reciprocal rows partition-broadcast by K=1 ones-matmuls,
                # then one DVE multiply per half.
                jsl = ds(j * CHUNK, CHUNK)
                av_sb = sbs[(j, hp)]
                _, rr2 = dens[j]
                for half in (0, 1):
                    src = rr2[0:1, hp, :] if half == 0 else rsts[(j, hp)][:]
                    bcp = psmm.tile([P, CHUNK], fp32, tag="psa", bufs=2)
                    nc.tensor.matmul(bcp[0:DK, :], ones_bf[0:1, 0:DK],
                                     src, start=True, stop=True)
                    if half == 0:
                        nc.vector.tensor_mul(hcat[0:DK, hp, jsl],
                                             av_sb[0:DK, 0, :], bcp[0:DK, :])
                    else:
                        ot = op_.tile([DK, CHUNK], bf16, tag="ot")
                        nc.vector.tensor_mul(ot[:], av_sb[0:DK, 1, :],
                                             bcp[0:DK, :])
                        nc.gpsimd.dma_start(hcat[DK:P, hp, jsl], ot[:])

            # ---------------- out-projection as filler units
            def outproj_units(j, copy_eng="dve"):
                units = []

                def u(st, nh):
                    def f():
                        osb = ob.tile([P, CHUNK], bf16, tag="osb",
                                      name="osb")
                        ps = psmm.tile([P, CHUNK], fp32, tag="psa", bufs=2)
                        for k2 in range(KC2):
                            nc.tensor.matmul(
                                ps[:], hcat[:, k2, ts(st, P)],
                                wo[:, k2, ds(nh * CHUNK, CHUNK)],
                                start=(k2 == 0), stop=(k2 == KC2 - 1))
                        use_act = (copy_eng == "act" or
                                   (copy_eng == "mix" and nh == 1))
                        if use_act:
                            nc.scalar.activation(osb[:], ps[:], Act.Copy)
                        else:
                            nc.vector.tensor_copy(osb[:], ps[:])
                        nc.gpsimd.dma_start(
                            outp[ts(st, P), ds(nh * CHUNK, CHUNK)], osb[:])
                    return f

                for st in range(j * TPC, (j + 1) * TPC):
                    for nh in range(NOC):
                        units.append(u(st, nh))
                return units

            # ---------------- emission
            from collections import deque
            filler = deque()
            gplan = []
            for j in range(NCH):
                nkv_j = min(TPC * (j + 1), NKV) if causal else NKV
                for hp in range(MT):
                    for i in range(nkv_j):
                        gplan.append((j, hp, i))

            dummy_mms(8)
            kq_units, v_units = make_stage_a_units(0)
            for u in kq_units:
                u()
            issue_scores(*gplan[0])
            for u in v_units:
                u()

            cur_j = -1
            avts = {}
            reserve = []
            for gstep, (j, hp, i) in enumerate(gplan):
                if j != cur_j:
                    cur_j = j
                    prefetch_x(j + 2)
                    if j >= 1:
                        for hp2 in range(MT):
                            phase2(j - 1, hp2)
                    if j + 1 < NCH:
                        kq, vs = make_stage_a_units(j + 1)
                        filler.extend(kq)
                        filler.extend(vs)
                    if j >= 1:
                        ou = outproj_units(j - 1)
                        if j == NCH - 1:
                            # hold back two units to bridge the final
                            # normalize -> out-projection bubble
                            reserve = ou[-2:]
                            ou = ou[:-2]
                        filler.extend(ou)
                nkv_j = min(TPC * (j + 1), NKV) if causal else NKV

                issue_exp(hp, i)
                if gstep + 1 < len(gplan):
                    issue_scores(*gplan[gstep + 1])
                # pop at most one ~1us filler unit per step: keeps the PE
                # fed through the exp tail without backing up the queue.
                if filler:
                    filler.popleft()()
                if i == 0:
                    pe = psav.tile([DK + 1, CHUNK], fp32, tag="av_e", bufs=1)
                    po = psav.tile([DK + 1, CHUNK], fp32, tag="av_o", bufs=1)
                    avts[hp] = (pe, po)
                pe, po = avts[hp]
                issue_av(hp, i, i == 0, i == nkv_j - 1, pe, po)
                if i == nkv_j - 1:
                    phase1(j, hp, pe, po)
                    if j == NCH - 1 and hp == 0:
                        # last chunk: hp0's normalize can complete while
                        # hp1 attention still streams
                        phase2(j, 0)

            # ---------------- epilogue
            while filler:
                filler.popleft()()
            for u in reserve:
                u()
            phase2(NCH - 1, MT - 1)
            for u in outproj_units(NCH - 1, copy_eng="mix"):
                u()

    nc.compile()
    return nc


def _gate(query, key_, Wg, bg):
    pooled = np.concatenate(
        [np.asarray(query, np.float64).mean(axis=1),
         np.asarray(key_, np.float64).mean(axis=1)], axis=-1)
    logits = pooled @ np.asarray(Wg, np.float64) + np.asarray(bg, np.float64)
    return 1.0 / (1.0 + np.exp(-logits))  # (B, H)


def _prep_core_inputs(shared, Wq, bq, Wk, bk, Wv, bv, Wo, g, b, gidx,
                      S, D, HPC, DK, with_bias):
    import ml_dtypes
    GCOLS = HPC * DK
    H0 = gidx * HPC
    cs = slice(H0 * DK, H0 * DK + GCOLS)
    f32 = np.float32
    bf16 = ml_dtypes.bfloat16
    c = np.ascontiguousarray
    g_rows = np.repeat(g[b, H0 : H0 + HPC], DK)[:, None]
    mtri = np.triu(np.ones((P, P), np.float32))
    d = {
        "xqT": shared["xqT"][b],
        "xkT": shared["xkT"][b],
        "xvT": shared["xvT"][b],
        "wq": c(Wq[:, cs].astype(bf16)),
        "wk": c(Wk[:, cs].astype(bf16)),
        "wv": c(Wv[:, cs].astype(bf16)),
        "wo": c((Wo[cs, :] * g_rows).astype(bf16)),
        "mtri": c(np.stack([mtri, mtri], axis=1).astype(bf16)),
    }
    if with_bias:
        d["bq"] = c(bq[cs].astype(f32))
        d["bk"] = c(bk[cs].astype(f32))
        d["bv"] = c(bv[cs].astype(bf16)[None, :])
    return d


_last_results = None


def kernel(query, key_, value, mask, Wq, bq, Wk, bk, Wv, bv, Wo, bo, Wg, bg):
    global _last_results
    import ml_dtypes
    from concourse.bass_utils import run_bass_kernel_spmd

    query = np.asarray(query)
    key_ = np.asarray(key_)
    value = np.asarray(value)
    mask = np.asarray(mask)
    B, S, D = query.shape
    H = np.asarray(bg).shape[0]
    DK = D // H
    DOUT = np.asarray(Wo).shape[1]
    NC_ = 8
    GROUPS = NC_ // B
    HPC = H // GROUPS

    causal = bool(
        np.array_equal(mask[0, 0], np.tril(np.ones((S, S), bool)))
    )
    if not causal:
        assert mask.all(), "only causal or all-true masks supported"

    with_bias = not (
        np.all(np.asarray(bq) == 0) and np.all(np.asarray(bk) == 0)
        and np.all(np.asarray(bv) == 0)
    )

    key = (S, D, DOUT, HPC, DK, causal, with_bias)
    if key not in _BUILD_CACHE:
        _BUILD_CACHE[key] = _build(*key)
    nc = _BUILD_CACHE[key]

    bf16 = ml_dtypes.bfloat16
    c = np.ascontiguousarray
    shared = {
        "xqT": [c(query[b].T.astype(bf16)) for b in range(B)],
        "xkT": [c(key_[b].T.astype(bf16)) for b in range(B)],
        "xvT": [c(value[b].T.astype(bf16)) for b in range(B)],
    }
    g = _gate(query, key_, Wg, bg)

    in_maps = []
    for cc in range(NC_):
        b, gidx = divmod(cc, GROUPS)
        in_maps.append(_prep_core_inputs(
            shared, Wq, bq, Wk, bk, Wv, bv, Wo, g, b, gidx, S, D, HPC, DK,
            with_bias))

    res = run_bass_kernel_spmd(nc, in_maps, core_ids=list(range(NC_)))
    _last_results = res

    out = np.zeros((B, S, DOUT), np.float32)
    for cc in range(NC_):
        b = cc // GROUPS
        out[b] += res.results[cc]["out"].astype(np.float32)
    out += np.asarray(bo).astype(np.float32)
    return out
